# revision 43
# baseline (speedup 1.0000x reference)
"""Trainium2 Bass kernel for nn_BDToGEConverter.

Computes out[b,s,p,d] = sum_k W_proj[p,d,k] * x_bd[b,s,k] for the fixed
sparse BD->GE projection W_proj [8,160,512]. Per (b,s) row, the 1280-col
output (viewed as 8 blocks of 160) is zero except:
  col 0   = sum_j j*x[64+j]      col 1   = sum_j j*x[96+j]
  col 160 = sum_j j*x[80+j]      col 161 = sum_j j*x[112+j]
  cols p*160 + 25..34 (all p)    = x[[16,17,10,11,12,13,14,15,18,19]]

Strategy: batch-parallel over 8 NeuronCores (core c handles batch c).
Each core reads only x[:, 0:128] (all needed columns live there),
assembles dense 1280-col output rows in persistent SBUF buffers whose
zero columns are memset once, and streams them out as large contiguous
DMA descriptors (scattered sub-512B HBM writes pay a ~70ns/descriptor
read-modify-write penalty on the SDMA engines, so dense stores win).
Per-core traffic is ~23MB, which runs at the per-NC HBM limit.
"""

import numpy as np

B, S, K = 8, 4096, 512
GE_DIM, P_BLK = 160, 8
D_OUT = P_BLK * GE_DIM  # 1280
ROWS = S  # rows handled per core (batch-parallel)
NIB_A, NIB_B, OP_START = 0, 1, 2
ALU_LO, ALU_HI = 64, 80
AX_CARRY_LO, AX_CARRY_HI = 96, 112
OPCODE_MAP = [(10, 25), (11, 26), (12, 27), (13, 28), (14, 29),
              (15, 30), (16, 23), (17, 24), (18, 31), (19, 32)]
# x columns feeding out cols p*160 + (25..34), in destination order:
SRC_ORDER = [16, 17, 10, 11, 12, 13, 14, 15, 18, 19]

# Tile schedule in units of 128 rows (rows per partition per tile). Small
# tiles at the start let the first store launch early; small tiles at the
# end shrink the final store's drain tail. Must sum to ROWS // 128 == 32.
T_SCHED = [2, 4, 4, 4, 4, 4, 4, 4, 1, 1]
T_MAX = max(T_SCHED)
N_TILES = len(T_SCHED)
# Single-assignment SBUF slots for the small tiles (xt/tmp/r): written once,
# so those DMAs never need WAW waits. The big dense out buffers are
# persistent (pre-zeroed once) and rotate through N_GBUF slots.
N_BUFS = N_TILES
N_GBUF = 6

_CACHE = {}


def _expected_w() -> np.ndarray:
    W = np.zeros((P_BLK, GE_DIM, K), dtype=np.float32)
    k = np.arange(16, dtype=np.float32)
    W[0, NIB_A, ALU_LO:ALU_LO + 16] = k
    W[0, NIB_B, AX_CARRY_LO:AX_CARRY_LO + 16] = k
    W[1, NIB_A, ALU_HI:ALU_HI + 16] = k
    W[1, NIB_B, AX_CARRY_HI:AX_CARRY_HI + 16] = k
    for pos in range(P_BLK):
        for bd_idx, ge_op in OPCODE_MAP:
            W[pos, OP_START + ge_op, bd_idx] = 1.0
    return W


def _build_nc():
    import concourse.bacc as bacc
    import concourse.mybir as mybir
    import concourse.tile as tile

    f32 = mybir.dt.float32
    # Bacc (not plain Bass): its finalize() runs generate_event_semaphores,
    # which legalizes multi-wait instructions for TRN2 (HW allows one sync
    # wait per instruction).
    nc = bacc.Bacc(trn_type="TRN2", name="bd_to_ge")
    x = nc.dram_tensor("x", [ROWS, K], f32, kind="ExternalInput")
    kv_in = nc.dram_tensor("kvec", [128, T_MAX * 64], f32, kind="ExternalInput")
    y = nc.dram_tensor("y", [ROWS, D_OUT], f32, kind="ExternalOutput")

    # Within a tile of t*128 rows, row r lives at partition r // t, slot
    # r % t; with the "(p t)" split each partition's slice of the output
    # tile is one contiguous t*5120B range of y, so a full-tile store is
    # 128 large descriptors — sub-512B scattered HBM writes cost ~70ns of
    # SDMA time each (read-modify-write), which made sparse stores the
    # bottleneck. We instead store dense rows from persistent buffers
    # whose zero columns are memset once and never touched again.
    with tile.TileContext(nc) as tc:
        with tc.tile_pool(name="const", bufs=1) as cpool, \
             tc.tile_pool(name="work", bufs=N_BUFS) as pool:
            kv = cpool.tile([128, T_MAX, 64], f32)
            nc.gpsimd.dma_start(out=kv[:], in_=kv_in.rearrange("p (t k) -> p t k", k=64))
            g_bufs = [
                cpool.tile([128, T_MAX, P_BLK, GE_DIM], f32, name=f"gb{b}")
                for b in range(N_GBUF)
            ]
            r0 = 0
            for i, t in enumerate(T_SCHED):
                if i < N_GBUF:
                    # one-time zero fill of just the slice this tile reads,
                    # so the first stores aren't delayed behind full
                    # memsets; the rest of the buffer is zeroed a few
                    # tiles later (see below). Value columns get
                    # overwritten on each use.
                    nc.vector.memset(g_bufs[i][:, 0:t], 0.0)
                nrows = 128 * t
                xi = x[r0:r0 + nrows, 0:128].rearrange(
                    "(p t) k -> p t k", p=128, t=t)
                yi = y[r0:r0 + nrows, :].rearrange(
                    "(p t) (pb c) -> p t pb c", p=128, t=t, pb=P_BLK)
                r0 += nrows

                xt = pool.tile([128, t, 128], f32, name=f"xt{i}", tag="xt")
                nc.gpsimd.dma_start(out=xt[:], in_=xi)

                # g[p, t, pb, :] = [nib0, nib1, 0*23, x[[16,17,10..15,18,19]], 0*125]
                g = g_bufs[i % N_GBUF][:, 0:t]
                nc.vector.tensor_copy(out=g[:, :, 0, 25:27], in_=xt[:, :, 16:18])
                nc.vector.tensor_copy(out=g[:, :, 0, 27:33], in_=xt[:, :, 10:16])
                nc.vector.tensor_copy(out=g[:, :, 0, 33:35], in_=xt[:, :, 18:20])
                # keep ACT free of ACTIVATE ops: any scalar-engine compute
                # would pull in ~6us of activation-table TENSOR_LOADs at
                # kernel start, delaying the first store issue
                nc.vector.tensor_copy(out=g[:, :, 1, 25:35], in_=g[:, :, 0, 25:35])
                nc.vector.tensor_copy(out=g[:, :, 2:4, 25:35], in_=g[:, :, 0:2, 25:35])
                nc.vector.tensor_copy(out=g[:, :, 4:8, 25:35], in_=g[:, :, 0:4, 25:35])

                # nibble-decode weighted sums over x[:, 64:128] in 4 groups of 16
                tmp = pool.tile([128, t, 64], f32, name=f"tmp{i}", tag="tmp")
                nc.vector.tensor_mul(out=tmp[:], in0=xt[:, :, 64:128], in1=kv[:, 0:t])
                r = pool.tile([128, t, 4], f32, name=f"r{i}", tag="r")
                nc.vector.reduce_sum(
                    out=r[:],
                    in_=tmp[:].rearrange("p t (g k) -> p t g k", k=16),
                    axis=mybir.AxisListType.X)

                # r groups (x-col order): [c0, c160, c1, c161] ->
                # g[pb=0, 0:2] = (r0, r2), g[pb=1, 0:2] = (r1, r3)
                rv = r[:].rearrange("p t (d g) -> p t d g", g=2)
                nc.vector.tensor_copy(out=g[:, :, 0, 0:2], in_=rv[:, :, :, 0])
                nc.vector.tensor_copy(out=g[:, :, 1, 0:2], in_=rv[:, :, :, 1])

                # Alternate the two HWDGE rings (SP / ACT)
                eng = nc.scalar if i % 2 == 0 else nc.sync
                eng.dma_start(out=yi[:], in_=g[:])

                if i >= 2:
                    # deferred complement memsets, once the pipeline is warm
                    b = i - 2
                    if b < N_GBUF and T_SCHED[b] < T_MAX:
                        nc.vector.memset(g_bufs[b][:, T_SCHED[b]:T_MAX], 0.0)
    nc.finalize()
    return nc


def _kvec_from_w(W: np.ndarray) -> np.ndarray:
    kvec = np.concatenate([
        W[0, NIB_A, ALU_LO:ALU_LO + 16],
        W[1, NIB_A, ALU_HI:ALU_HI + 16],
        W[0, NIB_B, AX_CARRY_LO:AX_CARRY_LO + 16],
        W[1, NIB_B, AX_CARRY_HI:AX_CARRY_HI + 16],
    ]).astype(np.float32)
    return np.tile(kvec, (128, T_MAX))


def kernel(x_bd, W_proj):
    x = np.ascontiguousarray(np.asarray(x_bd), dtype=np.float32)
    W = np.ascontiguousarray(np.asarray(W_proj), dtype=np.float32)
    if x.shape != (B, S, K) or W.shape != (P_BLK, GE_DIM, K) or \
            not np.array_equal(W, _expected_w()):
        # Generic (never expected) fallback: dense projection on host.
        out = x.reshape(-1, x.shape[-1]) @ W.reshape(-1, W.shape[-1]).T
        return np.ascontiguousarray(
            out.reshape(x.shape[0], x.shape[1], W.shape[0], W.shape[1]))

    import os

    from concourse.bass_utils import run_bass_kernel_spmd

    if "nc" not in _CACHE:
        _CACHE["nc"] = _build_nc()
    nc = _CACHE["nc"]

    kv_in = _kvec_from_w(W)
    in_maps = [{"x": x[c], "kvec": kv_in} for c in range(B)]
    try:
        try:
            res = run_bass_kernel_spmd(nc, in_maps, core_ids=list(range(B)))
        except ModuleNotFoundError:
            # BASS_TRACE=1 under axon needs antenv.axon_hooks, which some
            # images lack; retry with tracing disabled.
            os.environ["BASS_NEVER_TRACE"] = "1"
            res = run_bass_kernel_spmd(nc, in_maps, core_ids=list(range(B)))
    except Exception:
        # Transient device failures (e.g. NRT_EXEC_UNIT_UNRECOVERABLE):
        # one retry on device, then fall back to a correct host compute.
        try:
            res = run_bass_kernel_spmd(nc, in_maps, core_ids=list(range(B)))
        except Exception:
            out = x.reshape(-1, K) @ W.reshape(-1, K).T
            return np.ascontiguousarray(
                out.reshape(B, S, P_BLK, GE_DIM))
    _CACHE["last_results"] = res
    out = np.stack([res.results[c]["y"] for c in range(B)], axis=0)
    return out.reshape(B, S, P_BLK, GE_DIM)


# revision 44
# speedup vs baseline: 1.0033x; 1.0033x over previous
"""Trainium2 Bass kernel for nn_BDToGEConverter.

Computes out[b,s,p,d] = sum_k W_proj[p,d,k] * x_bd[b,s,k] for the fixed
sparse BD->GE projection W_proj [8,160,512]. Per (b,s) row, the 1280-col
output (viewed as 8 blocks of 160) is zero except:
  col 0   = sum_j j*x[64+j]      col 1   = sum_j j*x[96+j]
  col 160 = sum_j j*x[80+j]      col 161 = sum_j j*x[112+j]
  cols p*160 + 25..34 (all p)    = x[[16,17,10,11,12,13,14,15,18,19]]

Strategy: batch-parallel over 8 NeuronCores (core c handles batch c).
Each core reads only x[:, 0:128] (all needed columns live there),
assembles dense 1280-col output rows in persistent SBUF buffers whose
zero columns are memset once, and streams them out as large contiguous
DMA descriptors (scattered sub-512B HBM writes pay a ~70ns/descriptor
read-modify-write penalty on the SDMA engines, so dense stores win).
Per-core traffic is ~23MB, which runs at the per-NC HBM limit.
"""

import numpy as np

B, S, K = 8, 4096, 512
GE_DIM, P_BLK = 160, 8
D_OUT = P_BLK * GE_DIM  # 1280
ROWS = S  # rows handled per core (batch-parallel)
NIB_A, NIB_B, OP_START = 0, 1, 2
ALU_LO, ALU_HI = 64, 80
AX_CARRY_LO, AX_CARRY_HI = 96, 112
OPCODE_MAP = [(10, 25), (11, 26), (12, 27), (13, 28), (14, 29),
              (15, 30), (16, 23), (17, 24), (18, 31), (19, 32)]
# x columns feeding out cols p*160 + (25..34), in destination order:
SRC_ORDER = [16, 17, 10, 11, 12, 13, 14, 15, 18, 19]

# Tile schedule in units of 128 rows (rows per partition per tile). Small
# tiles at the start let the first store launch early; small tiles at the
# end shrink the final store's drain tail. Must sum to ROWS // 128 == 32.
T_SCHED = [2, 4, 4, 4, 4, 4, 4, 4, 1, 1]
T_MAX = max(T_SCHED)
N_TILES = len(T_SCHED)
# Single-assignment SBUF slots for the small tiles (xt/tmp/r): written once,
# so those DMAs never need WAW waits. The big dense out buffers are
# persistent (pre-zeroed once) and rotate through N_GBUF slots.
N_BUFS = N_TILES
# 4 buffers, not more: each buffer costs a ~2-4us DVE memset during the
# pipeline ramp, and store-read completion beats the distance-4 reuse
# comfortably (~5us slack), so extra buffers only slow the ramp.
N_GBUF = 4

_CACHE = {}


def _expected_w() -> np.ndarray:
    W = np.zeros((P_BLK, GE_DIM, K), dtype=np.float32)
    k = np.arange(16, dtype=np.float32)
    W[0, NIB_A, ALU_LO:ALU_LO + 16] = k
    W[0, NIB_B, AX_CARRY_LO:AX_CARRY_LO + 16] = k
    W[1, NIB_A, ALU_HI:ALU_HI + 16] = k
    W[1, NIB_B, AX_CARRY_HI:AX_CARRY_HI + 16] = k
    for pos in range(P_BLK):
        for bd_idx, ge_op in OPCODE_MAP:
            W[pos, OP_START + ge_op, bd_idx] = 1.0
    return W


def _build_nc():
    import concourse.bacc as bacc
    import concourse.mybir as mybir
    import concourse.tile as tile

    f32 = mybir.dt.float32
    # Bacc (not plain Bass): its finalize() runs generate_event_semaphores,
    # which legalizes multi-wait instructions for TRN2 (HW allows one sync
    # wait per instruction).
    nc = bacc.Bacc(trn_type="TRN2", name="bd_to_ge")
    x = nc.dram_tensor("x", [ROWS, K], f32, kind="ExternalInput")
    kv_in = nc.dram_tensor("kvec", [128, T_MAX * 64], f32, kind="ExternalInput")
    y = nc.dram_tensor("y", [ROWS, D_OUT], f32, kind="ExternalOutput")

    # Within a tile of t*128 rows, row r lives at partition r // t, slot
    # r % t; with the "(p t)" split each partition's slice of the output
    # tile is one contiguous t*5120B range of y, so a full-tile store is
    # 128 large descriptors — sub-512B scattered HBM writes cost ~70ns of
    # SDMA time each (read-modify-write), which made sparse stores the
    # bottleneck. We instead store dense rows from persistent buffers
    # whose zero columns are memset once and never touched again.
    with tile.TileContext(nc) as tc:
        with tc.tile_pool(name="const", bufs=1) as cpool, \
             tc.tile_pool(name="work", bufs=N_BUFS) as pool:
            kv = cpool.tile([128, T_MAX, 64], f32)
            nc.gpsimd.dma_start(out=kv[:], in_=kv_in.rearrange("p (t k) -> p t k", k=64))
            g_bufs = [
                cpool.tile([128, T_MAX, P_BLK, GE_DIM], f32, name=f"gb{b}")
                for b in range(N_GBUF)
            ]
            r0 = 0
            for i, t in enumerate(T_SCHED):
                if i < N_GBUF:
                    # one-time zero fill of just the slice this tile reads,
                    # so the first stores aren't delayed behind full
                    # memsets; the rest of the buffer is zeroed a few
                    # tiles later (see below). Value columns get
                    # overwritten on each use.
                    nc.vector.memset(g_bufs[i][:, 0:t], 0.0)
                nrows = 128 * t
                xi = x[r0:r0 + nrows, 0:128].rearrange(
                    "(p t) k -> p t k", p=128, t=t)
                yi = y[r0:r0 + nrows, :].rearrange(
                    "(p t) (pb c) -> p t pb c", p=128, t=t, pb=P_BLK)
                r0 += nrows

                xt = pool.tile([128, t, 128], f32, name=f"xt{i}", tag="xt")
                nc.gpsimd.dma_start(out=xt[:], in_=xi)

                # g[p, t, pb, :] = [nib0, nib1, 0*23, x[[16,17,10..15,18,19]], 0*125]
                g = g_bufs[i % N_GBUF][:, 0:t]
                nc.vector.tensor_copy(out=g[:, :, 0, 25:27], in_=xt[:, :, 16:18])
                nc.vector.tensor_copy(out=g[:, :, 0, 27:33], in_=xt[:, :, 10:16])
                nc.vector.tensor_copy(out=g[:, :, 0, 33:35], in_=xt[:, :, 18:20])
                # keep ACT free of ACTIVATE ops: any scalar-engine compute
                # would pull in ~6us of activation-table TENSOR_LOADs at
                # kernel start, delaying the first store issue
                nc.vector.tensor_copy(out=g[:, :, 1, 25:35], in_=g[:, :, 0, 25:35])
                nc.vector.tensor_copy(out=g[:, :, 2:4, 25:35], in_=g[:, :, 0:2, 25:35])
                nc.vector.tensor_copy(out=g[:, :, 4:8, 25:35], in_=g[:, :, 0:4, 25:35])

                # nibble-decode weighted sums over x[:, 64:128] in 4 groups of 16
                tmp = pool.tile([128, t, 64], f32, name=f"tmp{i}", tag="tmp")
                nc.vector.tensor_mul(out=tmp[:], in0=xt[:, :, 64:128], in1=kv[:, 0:t])
                r = pool.tile([128, t, 4], f32, name=f"r{i}", tag="r")
                nc.vector.reduce_sum(
                    out=r[:],
                    in_=tmp[:].rearrange("p t (g k) -> p t g k", k=16),
                    axis=mybir.AxisListType.X)

                # r groups (x-col order): [c0, c160, c1, c161] ->
                # g[pb=0, 0:2] = (r0, r2), g[pb=1, 0:2] = (r1, r3)
                rv = r[:].rearrange("p t (d g) -> p t d g", g=2)
                nc.vector.tensor_copy(out=g[:, :, 0, 0:2], in_=rv[:, :, :, 0])
                nc.vector.tensor_copy(out=g[:, :, 1, 0:2], in_=rv[:, :, :, 1])

                # Alternate the two HWDGE rings (SP / ACT)
                eng = nc.scalar if i % 2 == 0 else nc.sync
                eng.dma_start(out=yi[:], in_=g[:])

                if i >= 2:
                    # deferred complement memsets, once the pipeline is warm
                    b = i - 2
                    if b < N_GBUF and T_SCHED[b] < T_MAX:
                        nc.vector.memset(g_bufs[b][:, T_SCHED[b]:T_MAX], 0.0)
    nc.finalize()
    return nc


def _kvec_from_w(W: np.ndarray) -> np.ndarray:
    kvec = np.concatenate([
        W[0, NIB_A, ALU_LO:ALU_LO + 16],
        W[1, NIB_A, ALU_HI:ALU_HI + 16],
        W[0, NIB_B, AX_CARRY_LO:AX_CARRY_LO + 16],
        W[1, NIB_B, AX_CARRY_HI:AX_CARRY_HI + 16],
    ]).astype(np.float32)
    return np.tile(kvec, (128, T_MAX))


def kernel(x_bd, W_proj):
    x = np.ascontiguousarray(np.asarray(x_bd), dtype=np.float32)
    W = np.ascontiguousarray(np.asarray(W_proj), dtype=np.float32)
    if x.shape != (B, S, K) or W.shape != (P_BLK, GE_DIM, K) or \
            not np.array_equal(W, _expected_w()):
        # Generic (never expected) fallback: dense projection on host.
        out = x.reshape(-1, x.shape[-1]) @ W.reshape(-1, W.shape[-1]).T
        return np.ascontiguousarray(
            out.reshape(x.shape[0], x.shape[1], W.shape[0], W.shape[1]))

    import os

    from concourse.bass_utils import run_bass_kernel_spmd

    if "nc" not in _CACHE:
        _CACHE["nc"] = _build_nc()
    nc = _CACHE["nc"]

    kv_in = _kvec_from_w(W)
    in_maps = [{"x": x[c], "kvec": kv_in} for c in range(B)]
    try:
        try:
            res = run_bass_kernel_spmd(nc, in_maps, core_ids=list(range(B)))
        except ModuleNotFoundError:
            # BASS_TRACE=1 under axon needs antenv.axon_hooks, which some
            # images lack; retry with tracing disabled.
            os.environ["BASS_NEVER_TRACE"] = "1"
            res = run_bass_kernel_spmd(nc, in_maps, core_ids=list(range(B)))
    except Exception:
        # Transient device failures (e.g. NRT_EXEC_UNIT_UNRECOVERABLE):
        # one retry on device, then fall back to a correct host compute.
        try:
            res = run_bass_kernel_spmd(nc, in_maps, core_ids=list(range(B)))
        except Exception:
            out = x.reshape(-1, K) @ W.reshape(-1, K).T
            return np.ascontiguousarray(
                out.reshape(B, S, P_BLK, GE_DIM))
    _CACHE["last_results"] = res
    out = np.stack([res.results[c]["y"] for c in range(B)], axis=0)
    return out.reshape(B, S, P_BLK, GE_DIM)


# revision 46
# speedup vs baseline: 1.0180x; 1.0146x over previous
"""Trainium2 Bass kernel for nn_BDToGEConverter.

Computes out[b,s,p,d] = sum_k W_proj[p,d,k] * x_bd[b,s,k] for the fixed
sparse BD->GE projection W_proj [8,160,512]. Per (b,s) row, the 1280-col
output (viewed as 8 blocks of 160) is zero except:
  col 0   = sum_j j*x[64+j]      col 1   = sum_j j*x[96+j]
  col 160 = sum_j j*x[80+j]      col 161 = sum_j j*x[112+j]
  cols p*160 + 25..34 (all p)    = x[[16,17,10,11,12,13,14,15,18,19]]

Strategy: batch-parallel over 8 NeuronCores (core c handles batch c).
Each core reads only x[:, 0:128] (all needed columns live there),
assembles dense 1280-col output rows in persistent SBUF buffers whose
zero columns are memset once, and streams them out as large contiguous
DMA descriptors (scattered sub-512B HBM writes pay a ~70ns/descriptor
read-modify-write penalty on the SDMA engines, so dense stores win).
Per-core traffic is ~23MB, which runs at the per-NC HBM limit.
"""

import numpy as np

B, S, K = 8, 4096, 512
GE_DIM, P_BLK = 160, 8
D_OUT = P_BLK * GE_DIM  # 1280
ROWS = S  # rows handled per core (batch-parallel)
NIB_A, NIB_B, OP_START = 0, 1, 2
ALU_LO, ALU_HI = 64, 80
AX_CARRY_LO, AX_CARRY_HI = 96, 112
OPCODE_MAP = [(10, 25), (11, 26), (12, 27), (13, 28), (14, 29),
              (15, 30), (16, 23), (17, 24), (18, 31), (19, 32)]
# x columns feeding out cols p*160 + (25..34), in destination order:
SRC_ORDER = [16, 17, 10, 11, 12, 13, 14, 15, 18, 19]

# Tile schedule in units of 128 rows (rows per partition per tile). Small
# tiles at the start let the first store launch early; small tiles at the
# end shrink the final store's drain tail. Must sum to ROWS // 128 == 32.
T_SCHED = [2, 4, 4, 4, 4, 4, 4, 4, 1, 1]
T_MAX = max(T_SCHED)
N_TILES = len(T_SCHED)
# Single-assignment SBUF slots for the small tiles (xt/tmp/r): written once,
# so those DMAs never need WAW waits. The big dense out buffers are
# persistent (pre-zeroed once) and rotate through N_GBUF slots.
N_BUFS = N_TILES
# 4 buffers, not more: each buffer costs a ~2-4us DVE memset during the
# pipeline ramp, and store-read completion beats the distance-4 reuse
# comfortably (~5us slack), so extra buffers only slow the ramp.
N_GBUF = 4

_CACHE = {}


def _expected_w() -> np.ndarray:
    W = np.zeros((P_BLK, GE_DIM, K), dtype=np.float32)
    k = np.arange(16, dtype=np.float32)
    W[0, NIB_A, ALU_LO:ALU_LO + 16] = k
    W[0, NIB_B, AX_CARRY_LO:AX_CARRY_LO + 16] = k
    W[1, NIB_A, ALU_HI:ALU_HI + 16] = k
    W[1, NIB_B, AX_CARRY_HI:AX_CARRY_HI + 16] = k
    for pos in range(P_BLK):
        for bd_idx, ge_op in OPCODE_MAP:
            W[pos, OP_START + ge_op, bd_idx] = 1.0
    return W


def _build_nc():
    import concourse.bacc as bacc
    import concourse.mybir as mybir
    import concourse.tile as tile

    f32 = mybir.dt.float32
    # Bacc (not plain Bass): its finalize() runs generate_event_semaphores,
    # which legalizes multi-wait instructions for TRN2 (HW allows one sync
    # wait per instruction).
    nc = bacc.Bacc(trn_type="TRN2", name="bd_to_ge")
    x = nc.dram_tensor("x", [ROWS, K], f32, kind="ExternalInput")
    kv_in = nc.dram_tensor("kvec", [128, T_MAX * 64], f32, kind="ExternalInput")
    y = nc.dram_tensor("y", [ROWS, D_OUT], f32, kind="ExternalOutput")

    # Within a tile of t*128 rows, row r lives at partition r // t, slot
    # r % t; with the "(p t)" split each partition's slice of the output
    # tile is one contiguous t*5120B range of y, so a full-tile store is
    # 128 large descriptors — sub-512B scattered HBM writes cost ~70ns of
    # SDMA time each (read-modify-write), which made sparse stores the
    # bottleneck. We instead store dense rows from persistent buffers
    # whose zero columns are memset once and never touched again.
    with tile.TileContext(nc) as tc:
        with tc.tile_pool(name="const", bufs=1) as cpool, \
             tc.tile_pool(name="work", bufs=N_BUFS) as pool:
            kv = cpool.tile([128, T_MAX, 64], f32)
            nc.gpsimd.dma_start(out=kv[:], in_=kv_in.rearrange("p (t k) -> p t k", k=64))
            g_bufs = [
                cpool.tile([128, T_MAX, P_BLK, GE_DIM], f32, name=f"gb{b}")
                for b in range(N_GBUF)
            ]
            r0 = 0
            for i, t in enumerate(T_SCHED):
                if i < N_GBUF:
                    # one-time zero fill of just the slice this tile reads,
                    # so the first stores aren't delayed behind full
                    # memsets; the rest of the buffer is zeroed a few
                    # tiles later (see below). Value columns get
                    # overwritten on each use.
                    nc.vector.memset(g_bufs[i][:, 0:t], 0.0)
                nrows = 128 * t
                xi = x[r0:r0 + nrows, 0:128].rearrange(
                    "(p t) k -> p t k", p=128, t=t)
                yi = y[r0:r0 + nrows, :].rearrange(
                    "(p t) (pb c) -> p t pb c", p=128, t=t, pb=P_BLK)
                r0 += nrows

                xt = pool.tile([128, t, 128], f32, name=f"xt{i}", tag="xt")
                nc.gpsimd.dma_start(out=xt[:], in_=xi)

                # g[p, t, pb, :] = [nib0, nib1, 0*23, x[[16,17,10..15,18,19]], 0*125]
                g = g_bufs[i % N_GBUF][:, 0:t]
                nc.vector.tensor_copy(out=g[:, :, 0, 25:27], in_=xt[:, :, 16:18])
                nc.vector.tensor_copy(out=g[:, :, 0, 27:33], in_=xt[:, :, 10:16])
                nc.vector.tensor_copy(out=g[:, :, 0, 33:35], in_=xt[:, :, 18:20])
                # keep ACT free of ACTIVATE ops: any scalar-engine compute
                # would pull in ~6us of activation-table TENSOR_LOADs at
                # kernel start, delaying the first store issue
                nc.vector.tensor_copy(out=g[:, :, 1, 25:35], in_=g[:, :, 0, 25:35])
                nc.vector.tensor_copy(out=g[:, :, 2:4, 25:35], in_=g[:, :, 0:2, 25:35])
                nc.vector.tensor_copy(out=g[:, :, 4:8, 25:35], in_=g[:, :, 0:4, 25:35])

                # nibble-decode weighted sums over x[:, 64:128] in 4 groups of 16
                tmp = pool.tile([128, t, 64], f32, name=f"tmp{i}", tag="tmp")
                nc.vector.tensor_mul(out=tmp[:], in0=xt[:, :, 64:128], in1=kv[:, 0:t])
                r = pool.tile([128, t, 4], f32, name=f"r{i}", tag="r")
                nc.vector.reduce_sum(
                    out=r[:],
                    in_=tmp[:].rearrange("p t (g k) -> p t g k", k=16),
                    axis=mybir.AxisListType.X)

                # r groups (x-col order): [c0, c160, c1, c161] ->
                # g[pb=0, 0:2] = (r0, r2), g[pb=1, 0:2] = (r1, r3)
                rv = r[:].rearrange("p t (d g) -> p t d g", g=2)
                nc.vector.tensor_copy(out=g[:, :, 0, 0:2], in_=rv[:, :, :, 0])
                nc.vector.tensor_copy(out=g[:, :, 1, 0:2], in_=rv[:, :, :, 1])

                # Alternate the two HWDGE rings (SP / ACT)
                eng = nc.scalar if i % 2 == 0 else nc.sync
                eng.dma_start(out=yi[:], in_=g[:])

                if i >= 2:
                    # deferred complement memsets, once the pipeline is warm
                    b = i - 2
                    if b < N_GBUF and T_SCHED[b] < T_MAX:
                        nc.vector.memset(g_bufs[b][:, T_SCHED[b]:T_MAX], 0.0)
    nc.finalize()
    return nc


def _kvec_from_w(W: np.ndarray) -> np.ndarray:
    kvec = np.concatenate([
        W[0, NIB_A, ALU_LO:ALU_LO + 16],
        W[1, NIB_A, ALU_HI:ALU_HI + 16],
        W[0, NIB_B, AX_CARRY_LO:AX_CARRY_LO + 16],
        W[1, NIB_B, AX_CARRY_HI:AX_CARRY_HI + 16],
    ]).astype(np.float32)
    return np.tile(kvec, (128, T_MAX))


def kernel(x_bd, W_proj):
    x = np.ascontiguousarray(np.asarray(x_bd), dtype=np.float32)
    W = np.ascontiguousarray(np.asarray(W_proj), dtype=np.float32)
    if x.shape != (B, S, K) or W.shape != (P_BLK, GE_DIM, K) or \
            not np.array_equal(W, _expected_w()):
        # Generic (never expected) fallback: dense projection on host.
        out = x.reshape(-1, x.shape[-1]) @ W.reshape(-1, W.shape[-1]).T
        return np.ascontiguousarray(
            out.reshape(x.shape[0], x.shape[1], W.shape[0], W.shape[1]))

    import os

    from concourse.bass_utils import run_bass_kernel_spmd

    if "nc" not in _CACHE:
        _CACHE["nc"] = _build_nc()
    nc = _CACHE["nc"]

    kv_in = _kvec_from_w(W)
    in_maps = [{"x": x[c], "kvec": kv_in} for c in range(B)]
    try:
        try:
            res = run_bass_kernel_spmd(nc, in_maps, core_ids=list(range(B)))
        except ModuleNotFoundError:
            # BASS_TRACE=1 under axon needs antenv.axon_hooks, which some
            # images lack; retry with tracing disabled.
            os.environ["BASS_NEVER_TRACE"] = "1"
            res = run_bass_kernel_spmd(nc, in_maps, core_ids=list(range(B)))
    except Exception:
        # Transient device failures (e.g. NRT_EXEC_UNIT_UNRECOVERABLE):
        # one retry on device, then fall back to a correct host compute.
        try:
            res = run_bass_kernel_spmd(nc, in_maps, core_ids=list(range(B)))
        except Exception:
            out = x.reshape(-1, K) @ W.reshape(-1, K).T
            return np.ascontiguousarray(
                out.reshape(B, S, P_BLK, GE_DIM))
    _CACHE["last_results"] = res
    out = np.stack([res.results[c]["y"] for c in range(B)], axis=0)
    return out.reshape(B, S, P_BLK, GE_DIM)


# revision 48
# speedup vs baseline: 1.0918x; 1.0725x over previous
"""Trainium2 Bass kernel for nn_BDToGEConverter.

Computes out[b,s,p,d] = sum_k W_proj[p,d,k] * x_bd[b,s,k] for the fixed
sparse BD->GE projection W_proj [8,160,512]. Per (b,s) row, the 1280-col
output (viewed as 8 blocks of 160) is zero except:
  col 0   = sum_j j*x[64+j]      col 1   = sum_j j*x[96+j]
  col 160 = sum_j j*x[80+j]      col 161 = sum_j j*x[112+j]
  cols p*160 + 25..34 (all p)    = x[[16,17,10,11,12,13,14,15,18,19]]

Strategy: batch-parallel over 8 NeuronCores (core c handles batch c).
Each core reads only x[:, 0:128] (all needed columns live there),
assembles dense 1280-col output rows in persistent SBUF buffers whose
zero columns are memset once, and streams them out as large contiguous
DMA descriptors (scattered sub-512B HBM writes pay a ~70ns/descriptor
read-modify-write penalty on the SDMA engines, so dense stores win).
Per-core traffic is ~23MB, which runs at the per-NC HBM limit.
"""

import numpy as np

B, S, K = 8, 4096, 512
GE_DIM, P_BLK = 160, 8
D_OUT = P_BLK * GE_DIM  # 1280
ROWS = S  # rows handled per core (batch-parallel)
NIB_A, NIB_B, OP_START = 0, 1, 2
ALU_LO, ALU_HI = 64, 80
AX_CARRY_LO, AX_CARRY_HI = 96, 112
OPCODE_MAP = [(10, 25), (11, 26), (12, 27), (13, 28), (14, 29),
              (15, 30), (16, 23), (17, 24), (18, 31), (19, 32)]
# x columns feeding out cols p*160 + (25..34), in destination order:
SRC_ORDER = [16, 17, 10, 11, 12, 13, 14, 15, 18, 19]

# Tile schedule in units of 128 rows (rows per partition per tile). Small
# tiles at the start let the first store launch early; small tiles at the
# end shrink the final store's drain tail. Must sum to ROWS // 128 == 32.
T_SCHED = [2, 4, 4, 4, 4, 4, 4, 4, 1, 1]
T_MAX = max(T_SCHED)
N_TILES = len(T_SCHED)
# Single-assignment SBUF slots for the small tiles (xt/tmp/r): written once,
# so those DMAs never need WAW waits. The big dense out buffers are
# persistent (pre-zeroed once) and rotate through N_GBUF slots.
N_BUFS = N_TILES
# 4 buffers, not more: each buffer costs a ~2-4us DVE memset during the
# pipeline ramp, and store-read completion beats the distance-4 reuse
# comfortably (~5us slack), so extra buffers only slow the ramp.
N_GBUF = 4

_CACHE = {}


def _expected_w() -> np.ndarray:
    W = np.zeros((P_BLK, GE_DIM, K), dtype=np.float32)
    k = np.arange(16, dtype=np.float32)
    W[0, NIB_A, ALU_LO:ALU_LO + 16] = k
    W[0, NIB_B, AX_CARRY_LO:AX_CARRY_LO + 16] = k
    W[1, NIB_A, ALU_HI:ALU_HI + 16] = k
    W[1, NIB_B, AX_CARRY_HI:AX_CARRY_HI + 16] = k
    for pos in range(P_BLK):
        for bd_idx, ge_op in OPCODE_MAP:
            W[pos, OP_START + ge_op, bd_idx] = 1.0
    return W


def _build_nc():
    import concourse.bacc as bacc
    import concourse.mybir as mybir
    import concourse.tile as tile

    f32 = mybir.dt.float32
    # Bacc (not plain Bass): its finalize() runs generate_event_semaphores,
    # which legalizes multi-wait instructions for TRN2 (HW allows one sync
    # wait per instruction).
    nc = bacc.Bacc(trn_type="TRN2", name="bd_to_ge")
    x = nc.dram_tensor("x", [ROWS, K], f32, kind="ExternalInput")
    kv_in = nc.dram_tensor("kvec", [128, T_MAX * 64], f32, kind="ExternalInput")
    y = nc.dram_tensor("y", [ROWS, D_OUT], f32, kind="ExternalOutput")

    # Within a tile of t*128 rows, row r lives at partition r // t, slot
    # r % t; with the "(p t)" split each partition's slice of the output
    # tile is one contiguous t*5120B range of y, so a full-tile store is
    # 128 large descriptors — sub-512B scattered HBM writes cost ~70ns of
    # SDMA time each (read-modify-write), which made sparse stores the
    # bottleneck. We instead store dense rows from persistent buffers
    # whose zero columns are memset once and never touched again.
    with tile.TileContext(nc) as tc:
        with tc.tile_pool(name="const", bufs=1) as cpool, \
             tc.tile_pool(name="work", bufs=N_BUFS) as pool:
            kv = cpool.tile([128, T_MAX, 64], f32)
            nc.gpsimd.dma_start(out=kv[:], in_=kv_in.rearrange("p (t k) -> p t k", k=64))
            g_bufs = [
                cpool.tile([128, T_MAX, P_BLK, GE_DIM], f32, name=f"gb{b}")
                for b in range(N_GBUF)
            ]
            r0 = 0
            for i, t in enumerate(T_SCHED):
                if i < N_GBUF:
                    # one-time zero fill of just the slice this tile reads,
                    # so the first stores aren't delayed behind full
                    # memsets; the rest of the buffer is zeroed a few
                    # tiles later (see below). Value columns get
                    # overwritten on each use.
                    nc.vector.memset(g_bufs[i][:, 0:t], 0.0)
                nrows = 128 * t
                xi = x[r0:r0 + nrows, 0:128].rearrange(
                    "(p t) k -> p t k", p=128, t=t)
                yi = y[r0:r0 + nrows, :].rearrange(
                    "(p t) (pb c) -> p t pb c", p=128, t=t, pb=P_BLK)
                r0 += nrows

                xt = pool.tile([128, t, 128], f32, name=f"xt{i}", tag="xt")
                nc.gpsimd.dma_start(out=xt[:], in_=xi)

                # g[p, t, pb, :] = [nib0, nib1, 0*23, x[[16,17,10..15,18,19]], 0*125]
                g = g_bufs[i % N_GBUF][:, 0:t]
                nc.vector.tensor_copy(out=g[:, :, 0, 25:27], in_=xt[:, :, 16:18])
                nc.vector.tensor_copy(out=g[:, :, 0, 27:33], in_=xt[:, :, 10:16])
                nc.vector.tensor_copy(out=g[:, :, 0, 33:35], in_=xt[:, :, 18:20])
                # keep ACT free of ACTIVATE ops: any scalar-engine compute
                # would pull in ~6us of activation-table TENSOR_LOADs at
                # kernel start, delaying the first store issue
                nc.vector.tensor_copy(out=g[:, :, 1, 25:35], in_=g[:, :, 0, 25:35])
                nc.vector.tensor_copy(out=g[:, :, 2:4, 25:35], in_=g[:, :, 0:2, 25:35])
                nc.vector.tensor_copy(out=g[:, :, 4:8, 25:35], in_=g[:, :, 0:4, 25:35])

                # nibble-decode weighted sums over x[:, 64:128] in 4 groups of 16
                tmp = pool.tile([128, t, 64], f32, name=f"tmp{i}", tag="tmp")
                nc.vector.tensor_mul(out=tmp[:], in0=xt[:, :, 64:128], in1=kv[:, 0:t])
                r = pool.tile([128, t, 4], f32, name=f"r{i}", tag="r")
                nc.vector.reduce_sum(
                    out=r[:],
                    in_=tmp[:].rearrange("p t (g k) -> p t g k", k=16),
                    axis=mybir.AxisListType.X)

                # r groups (x-col order): [c0, c160, c1, c161] ->
                # g[pb=0, 0:2] = (r0, r2), g[pb=1, 0:2] = (r1, r3)
                rv = r[:].rearrange("p t (d g) -> p t d g", g=2)
                nc.vector.tensor_copy(out=g[:, :, 0, 0:2], in_=rv[:, :, :, 0])
                nc.vector.tensor_copy(out=g[:, :, 1, 0:2], in_=rv[:, :, :, 1])

                # Alternate the two HWDGE rings (SP / ACT)
                eng = nc.scalar if i % 2 == 0 else nc.sync
                eng.dma_start(out=yi[:], in_=g[:])

                if i >= 2:
                    # deferred complement memsets, once the pipeline is warm
                    b = i - 2
                    if b < N_GBUF and T_SCHED[b] < T_MAX:
                        nc.vector.memset(g_bufs[b][:, T_SCHED[b]:T_MAX], 0.0)
    nc.finalize()
    return nc


def _kvec_from_w(W: np.ndarray) -> np.ndarray:
    kvec = np.concatenate([
        W[0, NIB_A, ALU_LO:ALU_LO + 16],
        W[1, NIB_A, ALU_HI:ALU_HI + 16],
        W[0, NIB_B, AX_CARRY_LO:AX_CARRY_LO + 16],
        W[1, NIB_B, AX_CARRY_HI:AX_CARRY_HI + 16],
    ]).astype(np.float32)
    return np.tile(kvec, (128, T_MAX))


def kernel(x_bd, W_proj):
    x = np.ascontiguousarray(np.asarray(x_bd), dtype=np.float32)
    W = np.ascontiguousarray(np.asarray(W_proj), dtype=np.float32)
    if x.shape != (B, S, K) or W.shape != (P_BLK, GE_DIM, K) or \
            not np.array_equal(W, _expected_w()):
        # Generic (never expected) fallback: dense projection on host.
        out = x.reshape(-1, x.shape[-1]) @ W.reshape(-1, W.shape[-1]).T
        return np.ascontiguousarray(
            out.reshape(x.shape[0], x.shape[1], W.shape[0], W.shape[1]))

    import os

    from concourse.bass_utils import run_bass_kernel_spmd

    if "nc" not in _CACHE:
        _CACHE["nc"] = _build_nc()
    nc = _CACHE["nc"]

    kv_in = _kvec_from_w(W)
    in_maps = [{"x": x[c], "kvec": kv_in} for c in range(B)]
    try:
        try:
            res = run_bass_kernel_spmd(nc, in_maps, core_ids=list(range(B)))
        except ModuleNotFoundError:
            # BASS_TRACE=1 under axon needs antenv.axon_hooks, which some
            # images lack; retry with tracing disabled.
            os.environ["BASS_NEVER_TRACE"] = "1"
            res = run_bass_kernel_spmd(nc, in_maps, core_ids=list(range(B)))
    except Exception:
        # Transient device failures (e.g. NRT_EXEC_UNIT_UNRECOVERABLE):
        # one retry on device, then fall back to a correct host compute.
        try:
            res = run_bass_kernel_spmd(nc, in_maps, core_ids=list(range(B)))
        except Exception:
            out = x.reshape(-1, K) @ W.reshape(-1, K).T
            return np.ascontiguousarray(
                out.reshape(B, S, P_BLK, GE_DIM))
    _CACHE["last_results"] = res
    out = np.stack([res.results[c]["y"] for c in range(B)], axis=0)
    return out.reshape(B, S, P_BLK, GE_DIM)
